# revision 2
# baseline (speedup 1.0000x reference)
"""AWQ linear (int4 group-quantized) matmul on 8 Trainium2 NeuronCores.

out[m, n] = sum_k x[m, k] * W[n, k] + bias[n]
W[n, k] = (q4[n, k] - qzeros[n, k//128]) * qscales[n, k//128]

Strategy (column-parallel): shard N=11008 across 8 cores (1376 each),
replicate x. Per core:
  - host repacks qweight nibbles to a k-major uint8 tensor [K, Ns]
  - device dequantizes W^T[k, n] = (q4 - z)*s into resident SBUF bf16
    using DMA-broadcast scale/zero rows (exact fp32 affine, bf16 store)
  - x^T (bf16, host-swizzled into [128, K] m-tiles) is the matmul
    stationary operand; W^T tiles stream; PSUM accumulates over k
  - bias is fused into the PSUM->SBUF eviction
"""

import os

import numpy as np
import ml_dtypes

M, K, NFULL = 4096, 4096, 11008
NCORES = 8
NS = NFULL // NCORES          # 1376 out-features per core
P = 128                       # partitions; also the quant group size
MM_FREE = 512                 # psum bank limit (fp32)

LAST_RESULTS = None           # BassKernelResults of the last kernel() call


def build_nc(k=K, m=M, ns=NS, n_cores=NCORES):
    """Build + compile the per-core Bass program (SPMD: same NEFF on all cores)."""
    import concourse.mybir as mybir
    import concourse.tile as tile
    from concourse import bacc

    kt_n = k // P
    mt_n = m // P
    chunks = [(i, min(MM_FREE, ns - i)) for i in range(0, ns, MM_FREE)]

    f32 = mybir.dt.float32
    bf16 = mybir.dt.bfloat16
    u8 = mybir.dt.uint8

    nc = bacc.Bacc("TRN2", num_devices=n_cores)
    xt = nc.dram_tensor("xt", [m, k], bf16, kind="ExternalInput")
    q4 = nc.dram_tensor("q4", [k, ns], u8, kind="ExternalInput")
    scl = nc.dram_tensor("scl", [kt_n, ns], f32, kind="ExternalInput")
    zro = nc.dram_tensor("zro", [kt_n, ns], f32, kind="ExternalInput")
    bias = nc.dram_tensor("bias", [1, ns], f32, kind="ExternalInput")
    out = nc.dram_tensor("out", [m, ns], f32, kind="ExternalOutput")

    with tile.TileContext(nc) as tc:
        with (
            tc.tile_pool(name="persist", bufs=1) as persist,
            tc.tile_pool(name="dq", bufs=2) as dq,
            tc.tile_pool(name="xp", bufs=3) as xp,
            tc.tile_pool(name="op", bufs=2) as op,
            tc.tile_pool(name="ps", bufs=4, space="PSUM") as ps,
        ):
            w_all = persist.tile([P, kt_n, ns], bf16)
            bias_exp = persist.tile([P, ns], f32)
            nc.sync.dma_start(bias_exp[:], bias.ap().to_broadcast((P, ns)))

            # Dequantize W^T into resident SBUF, one k-tile (= one group) at a time.
            for kt in range(kt_n):
                q4t = dq.tile([P, ns], u8, tag="q4t")
                nc.sync.dma_start(q4t[:], q4.ap()[kt * P:(kt + 1) * P, :])
                z_exp = dq.tile([P, ns], f32, tag="z_exp")
                nc.sync.dma_start(
                    z_exp[:], zro.ap()[kt:kt + 1, :].to_broadcast((P, ns))
                )
                s_exp = dq.tile([P, ns], f32, tag="s_exp")
                nc.sync.dma_start(
                    s_exp[:], scl.ap()[kt:kt + 1, :].to_broadcast((P, ns))
                )
                tmp = dq.tile([P, ns], f32, tag="tmp")
                nc.vector.tensor_tensor(
                    tmp[:], q4t[:], z_exp[:], mybir.AluOpType.subtract
                )
                nc.vector.tensor_tensor(
                    w_all[:, kt, :], tmp[:], s_exp[:], mybir.AluOpType.mult
                )

            # Main matmul: x^T tiles stationary, W^T tiles moving, PSUM over k.
            for mt in range(mt_n):
                xbf = xp.tile([P, kt_n * P], bf16, tag="xbf")
                nc.sync.dma_start(xbf[:], xt.ap()[mt * P:(mt + 1) * P, :])
                outsb = op.tile([P, ns], f32, tag="outsb")
                for nstart, sz in chunks:
                    pst = ps.tile([P, MM_FREE], f32, tag="psum")
                    for kt in range(kt_n):
                        nc.tensor.matmul(
                            pst[:, :sz],
                            xbf[:, kt * P:(kt + 1) * P],
                            w_all[:, kt, nstart:nstart + sz],
                            start=(kt == 0),
                            stop=(kt == kt_n - 1),
                        )
                    nc.vector.tensor_tensor(
                        outsb[:, nstart:nstart + sz],
                        pst[:, :sz],
                        bias_exp[:, nstart:nstart + sz],
                        mybir.AluOpType.add,
                    )
                nc.sync.dma_start(out.ap()[mt * P:(mt + 1) * P, :], outsb[:])

    nc.compile()
    return nc


def prep_inputs(x, qweight, qscales, qzeros, bias):
    """Host-side shard/layout prep. Returns per-core input maps."""
    x = np.asarray(x)
    qweight = np.asarray(qweight)
    qscales = np.asarray(qscales)
    qzeros = np.asarray(qzeros)
    bias = np.asarray(bias)

    kt_n = K // P
    mt_n = M // P

    # x^T in bf16, swizzled so each m-tile is one contiguous [128, K] slab:
    # xprep[mt*128 + p, kt*128 + j] = x[mt*128 + j, kt*128 + p]
    xbf = x.astype(ml_dtypes.bfloat16)
    xprep = np.ascontiguousarray(
        xbf.reshape(mt_n, P, kt_n, P).transpose(0, 3, 2, 1).reshape(M, K)
    )

    # Unpack int4 nibbles into k-major uint8 [K, N]:
    # even k -> low nibble, odd k -> high nibble of byte qweight[n, k//2]
    b = qweight.astype(np.uint8)              # [N, K//2]
    q4 = np.empty((K, NFULL), np.uint8)
    q4[0::2, :] = (b & 15).T
    q4[1::2, :] = (b >> 4).T

    sT = np.ascontiguousarray(qscales.astype(np.float32).T)   # [G, N]
    zT = np.ascontiguousarray(qzeros.astype(np.float32).T)    # [G, N]
    bias2d = bias.astype(np.float32).reshape(1, NFULL)

    in_maps = []
    for c in range(NCORES):
        sl = slice(c * NS, (c + 1) * NS)
        in_maps.append(
            {
                "xt": xprep,
                "q4": np.ascontiguousarray(q4[:, sl]),
                "scl": np.ascontiguousarray(sT[:, sl]),
                "zro": np.ascontiguousarray(zT[:, sl]),
                "bias": np.ascontiguousarray(bias2d[:, sl]),
            }
        )
    return in_maps


def kernel(x, qweight, qscales, qzeros, bias):
    global LAST_RESULTS
    from concourse.bass_utils import run_bass_kernel_spmd

    nc = build_nc()
    in_maps = prep_inputs(x, qweight, qscales, qzeros, bias)
    trace = bool(os.environ.get("BASS_AWQ_TRACE"))
    res = run_bass_kernel_spmd(
        nc,
        in_maps,
        core_ids=list(range(NCORES)),
        trace=trace,
        trace_cores=list(range(NCORES)) if trace else None,
    )
    LAST_RESULTS = res
    return np.concatenate([res.results[c]["out"] for c in range(NCORES)], axis=1)
